# revision 6
# baseline (speedup 1.0000x reference)
"""Trainium2 Bass kernel for nn_CPIPre (GNN message passing + conv/attention).

Strategy (8 NeuronCores, SPMD + collectives):
  - adjacency A [8192, 8192] row-sharded: core c owns rows [1024c, 1024(c+1)).
    Host ships A_blk^T ([8192, 1024], bf16) per core; it is DMA'd once into
    SBUF and stays resident for all 3 GNN layers (memory roofline = read A
    once at bf16).
  - Per GNN layer: every core computes hs = relu(xs@Wg^T+b) for its own rows
    (tensor engine, bias folded in via an ones-row), AllGathers hs (bf16,
    20KB/rank), then computes its row block of A@hs as 128 accumulating
    matmuls (hs tile stationary [128,10], A^T moving [128,512], 4-way
    column-tiled PE). xs stays fp32-resident per core.
  - The [1,2] result needs mean(xs3) only, so layer 3 skips the xs update and
    AllGathers per-core partial sums [10,1] instead.
  - Conv branch (11x11 conv on the [4096,10] word embedding "image") is
    reformulated as one K=111 matmul per 512-column tile against a stacked
    shifted-copies tensor S [111, 4096] built by 11 shift-DMAs; bias via an
    ones-row in S. Attention + output MLP are tiny K<=21 matmuls.
  - Everything except the A row-block and index gathers is replicated; the
    final output is read from core 0.
"""
import numpy as np
import ml_dtypes

N = 8192
D = 10
NCORES = 8
NB = N // NCORES          # 1024 rows per core
NT = N // 128             # 64 contraction tiles
L = 4096
KW = 11
PAD = 5
LW = L + 2 * PAD          # 4106 padded width
LG = LC = LO = 3

BF16_NP = ml_dtypes.bfloat16
FP8_NP = ml_dtypes.float8_e4m3

_CACHE = {}


def _build_nc(reps=1):
    import concourse.bacc as bacc
    import concourse.mybir as mybir
    from concourse import tile

    F32 = mybir.dt.float32
    BF16 = mybir.dt.bfloat16
    FP8 = mybir.dt.float8e4
    AF = mybir.ActivationFunctionType
    ALU = mybir.AluOpType
    AX = mybir.AxisListType

    nc = bacc.Bacc("TRN2", target_bir_lowering=False, debug=False,
                   num_devices=NCORES)

    at_d = nc.dram_tensor("AT", [N, NB], FP8, kind="ExternalInput")
    xs0_d = nc.dram_tensor("XS0", [D + 1, NB], F32, kind="ExternalInput")
    ws0_d = nc.dram_tensor("WS0", [D, LW], F32, kind="ExternalInput")
    wg_d = nc.dram_tensor("WG", [LG, D + 1, D], F32, kind="ExternalInput")
    hst_d = nc.dram_tensor("HST", [LC, KW * D + 1, D], F32, kind="ExternalInput")
    wa_d = nc.dram_tensor("WA", [D + 1, D], F32, kind="ExternalInput")
    wo_d = nc.dram_tensor("WO", [LO, 2 * D + 1, 2 * D], F32, kind="ExternalInput")
    wi_d = nc.dram_tensor("WI", [2 * D + 1, 2], F32, kind="ExternalInput")
    sum8_d = nc.dram_tensor("SUM8", [NCORES * D, D], F32, kind="ExternalInput")
    ones_d = nc.dram_tensor("ONES", [1, LW], F32, kind="ExternalInput")
    out_d = nc.dram_tensor("OUT", [2, 1], F32, kind="ExternalOutput")

    rg = [list(range(NCORES))]

    with tile.TileContext(nc) as tc:
        with (
            tc.tile_pool(name="const", bufs=1) as cp,
            tc.tile_pool(name="work", bufs=2) as wp,
            tc.tile_pool(name="pbig", bufs=2, space="PSUM") as pp_big,
            tc.tile_pool(name="psmall", bufs=2, space="PSUM") as pp_small,
            tc.tile_pool(name="pconv", bufs=2, space="PSUM") as pp_conv,
            tc.tile_pool(name="pattn", bufs=2, space="PSUM") as pp_attn,
            tc.tile_pool(name="dram", bufs=1, space="DRAM") as dp,
        ):
          for _rep in range(reps):
            # ---------------- A^T resident load (16 chunks of 4 k-tiles) ----
            at_sb = cp.tile([128, NT * NB], FP8)
            for g in range(16):
                nc.sync.dma_start(
                    at_sb[:, 4 * NB * g:4 * NB * (g + 1)]
                        .rearrange("p (t n) -> p t n", n=NB),
                    at_d[512 * g:512 * (g + 1), :]
                        .rearrange("(t p) n -> p t n", p=128),
                )

            # ---------------- constants ------------------------------------
            wg_sb = cp.tile([D + 1, LG * D], F32)
            nc.sync.dma_start(wg_sb.rearrange("k (i d) -> k i d", d=D),
                              wg_d.rearrange("i k d -> k i d"))
            hst_sb = cp.tile([KW * D + 1, LC * D], F32)
            nc.sync.dma_start(hst_sb.rearrange("k (i d) -> k i d", d=D),
                              hst_d.rearrange("i k d -> k i d"))
            wa_sb = cp.tile([D + 1, D], F32)
            nc.sync.dma_start(wa_sb[:, :], wa_d[:, :])
            wo_sb = cp.tile([2 * D + 1, LO * 2 * D], F32)
            nc.sync.dma_start(wo_sb.rearrange("k (i d) -> k i d", d=2 * D),
                              wo_d.rearrange("i k d -> k i d"))
            wi_sb = cp.tile([2 * D + 1, 2], F32)
            nc.sync.dma_start(wi_sb[:, :], wi_d[:, :])
            sum8_sb = cp.tile([NCORES * D, D], F32)
            nc.sync.dma_start(sum8_sb[:, :], sum8_d[:, :])
            ws0_sb = cp.tile([D, LW], F32)
            nc.sync.dma_start(ws0_sb[:, :], ws0_d[:, :])

            xs_cur = wp.tile([D + 1, NB], F32, name="xs0", tag="xs")
            nc.sync.dma_start(xs_cur[:, :], xs0_d[:, :])

            # conv buffers
            s_sb = cp.tile([KW * D + 1, L], F32)
            ws1_sb = cp.tile([D, LW], F32)
            ws2_sb = cp.tile([D, LW], F32)
            ws3_sb = cp.tile([D + 1, LW], F32)
            nc.vector.memset(ws1_sb[:, :], 0.0)
            nc.vector.memset(ws2_sb[:, :], 0.0)
            nc.sync.dma_start(s_sb[KW * D:KW * D + 1, :], ones_d[0:1, 0:L])
            nc.sync.dma_start(ws3_sb[D:D + 1, PAD:PAD + L], ones_d[0:1, 0:L])

            # attention / MLP buffers
            hsa_sb = cp.tile([D, L], F32)
            wt_sb = cp.tile([1, L], F32)
            ys_part = cp.tile([D, 8], F32)
            part9 = cp.tile([D, 9], F32)
            ppart = cp.tile([D, 1], F32)
            ppro = cp.tile([D, 1], F32)
            hv = cp.tile([D, 1], F32)
            cvec = cp.tile([D + 1, 1], F32)
            catv = cp.tile([2 * D + 1, LO + 1], F32)
            res_sb = cp.tile([2, 1], F32)
            ones10 = cp.tile([1, D], F32)
            nc.sync.dma_start(cvec[D:D + 1, :], ones_d[0:1, 0:1])
            nc.sync.dma_start(catv[2 * D:2 * D + 1, :], ones_d[0:1, 0:LO + 1])
            nc.vector.memset(ones10[:, :], 1.0)

            ws_srcs = [ws0_sb, ws1_sb, ws2_sb, ws3_sb]

            def conv_layer(i):
                src = ws_srcs[i]
                dst = ws_srcs[i + 1]
                for s in range(KW):
                    nc.sync.dma_start(s_sb[D * s:D * (s + 1), 0:L],
                                      src[0:D, s:s + L])
                for half in range(2):
                    pc = pp_conv.tile([128, 512], F32,
                                      name=f"ps_c{i}_{half}", tag="conv")
                    for q in range(4):
                        nt = 4 * half + q
                        nc.tensor.matmul(
                            pc[32 * q:32 * q + D, :],
                            hst_sb[:, D * i:D * (i + 1)],
                            s_sb[:, 512 * nt:512 * (nt + 1)],
                            start=True, stop=True,
                            tile_position=(0, 32 * q),
                        )
                    for q in range(4):
                        nt = 4 * half + q
                        nc.scalar.activation(
                            dst[0:D, PAD + 512 * nt:PAD + 512 * (nt + 1)],
                            pc[32 * q:32 * q + D, :], AF.Relu)

            # ---------------- GNN layers ------------------------------------
            for i in range(LG):
                # hs for my rows: 8 matmuls [128,10] + relu->bf16
                ps_s = pp_small.tile([128, 8 * D], F32,
                                     name=f"ps_s{i}", tag="small")
                for t in range(8):
                    nc.tensor.matmul(
                        ps_s[:, D * t:D * (t + 1)],
                        xs_cur[:, 128 * t:128 * (t + 1)],
                        wg_sb[:, D * i:D * (i + 1)],
                        start=True, stop=True)
                hsl = wp.tile([128, 8 * D], BF16, name=f"hsl{i}", tag="hsl")
                nc.scalar.activation(hsl[:, :], ps_s[:, :], AF.Relu)

                cc_in = dp.tile([128, 8 * D], BF16,
                                name=f"cci{i}", tag=f"cci{i}")
                nc.sync.dma_start(cc_in[:, :], hsl[:, :])
                cc_out = dp.tile([128 * NCORES, 8 * D], BF16,
                                 name=f"cco{i}", tag=f"cco{i}",
                                 addr_space="Shared")
                nc.gpsimd.collective_compute(
                    "AllGather", ALU.bypass, replica_groups=rg,
                    ins=[cc_in.opt()], outs=[cc_out.opt()])
                hs_sb = wp.tile([128, NT * D], BF16, name=f"hs{i}", tag="hs")
                nc.sync.dma_start(
                    hs_sb.rearrange("p (r f) -> p r f", f=8 * D),
                    cc_out.rearrange("(r p) f -> p r f", p=128))

                # conv layer emitted here to interleave PE work
                if i < LC:
                    conv_layer(i)

                # big matmul: out^T [10, 1024] += hs_kt^T @ A^T_kt, col-tiled
                ps_b = []
                for h in range(2):
                    pb = pp_big.tile([128, 512], F32,
                                     name=f"ps_b{i}_{h}", tag="big")
                    for kt in range(NT):
                        g = kt % 4
                        nc.tensor.matmul(
                            pb[32 * g:32 * g + D, :],
                            hs_sb[:, D * kt:D * (kt + 1)],
                            at_sb[:, NB * kt + 512 * h:NB * kt + 512 * h + 512],
                            start=(kt < 4), stop=(kt >= NT - 4),
                            tile_position=(0, 32 * g),
                        )
                    ps_b.append(pb)

                if i < LG - 1:
                    xs_next = wp.tile([D + 1, NB], F32,
                                      name=f"xs{i + 1}", tag="xs")
                    nc.sync.dma_start(xs_next[D:D + 1, :], ones_d[0:1, 0:NB])
                    for h in range(2):
                        sl = slice(512 * h, 512 * (h + 1))
                        tmp = wp.tile([D, 512], F32,
                                      name=f"upd{i}_{h}", tag="upd")
                        nc.vector.scalar_tensor_tensor(
                            tmp[:, :], xs_cur[0:D, sl], 0.0,
                            ps_b[h][0:D, :], ALU.add, ALU.add)
                        nc.vector.tensor_add(tmp[:, :], tmp[:, :],
                                             ps_b[h][32:32 + D, :])
                        nc.vector.tensor_add(tmp[:, :], tmp[:, :],
                                             ps_b[h][64:64 + D, :])
                        nc.vector.tensor_add(xs_next[0:D, sl], tmp[:, :],
                                             ps_b[h][96:96 + D, :])
                    xs_cur = xs_next
                else:
                    # partial compound sums: sum over my rows of xs2 + A@hs2
                    nc.vector.reduce_sum(part9[:, 0:1], xs_cur[0:D, :],
                                         axis=AX.X)
                    for h in range(2):
                        for g in range(4):
                            col = 1 + 4 * h + g
                            nc.vector.reduce_sum(
                                part9[:, col:col + 1],
                                ps_b[h][32 * g:32 * g + D, :], axis=AX.X)
                    nc.vector.reduce_sum(ppart[:, :], part9[:, :], axis=AX.X)

            # ---------------- compound via tiny AllGather -------------------
            cc2_in = dp.tile([D, 1], F32, name="cc2i", tag="cc2i")
            nc.sync.dma_start(cc2_in[:, :], ppart[:, :])
            cc2_out = dp.tile([NCORES * D, 1], F32, name="cc2o", tag="cc2o",
                              addr_space="Shared")
            nc.gpsimd.collective_compute(
                "AllGather", ALU.bypass, replica_groups=rg,
                ins=[cc2_in.opt()], outs=[cc2_out.opt()])
            pc_sb = cp.tile([NCORES * D, 1], F32)
            nc.sync.dma_start(pc_sb[:, :], cc2_out[:, :])
            ps_cmp = pp_attn.tile([D, 1], F32, name="ps_cmp", tag="attn")
            nc.tensor.matmul(ps_cmp[:, :], sum8_sb[:, :], pc_sb[:, :],
                             start=True, stop=True)
            nc.scalar.activation(cvec[0:D, :], ps_cmp[:, :], AF.Copy,
                                 scale=1.0 / N)
            nc.scalar.activation(catv[0:D, 0:1], ps_cmp[:, :], AF.Copy,
                                 scale=1.0 / N)

            # ---------------- attention ------------------------------------
            for nt in range(8):
                pa = pp_attn.tile([D, 512], F32, name=f"ps_a{nt}", tag="attn")
                nc.tensor.matmul(pa[:, :], wa_sb[:, :],
                                 ws3_sb[:, PAD + 512 * nt:PAD + 512 * (nt + 1)],
                                 start=True, stop=True)
                nc.scalar.activation(hsa_sb[:, 512 * nt:512 * (nt + 1)],
                                     pa[:, :], AF.Relu)
            ph = pp_attn.tile([D, 1], F32, name="ps_h", tag="attn")
            nc.tensor.matmul(ph[:, :], wa_sb[:, :], cvec[:, :],
                             start=True, stop=True)
            nc.scalar.activation(hv[:, :], ph[:, :], AF.Relu)

            ys_scr = cp.tile([D, 512], F32)
            for nt in range(8):
                sl = slice(512 * nt, 512 * (nt + 1))
                pw = pp_attn.tile([1, 512], F32, name=f"ps_w{nt}", tag="attn")
                nc.tensor.matmul(pw[:, :], hv[:, :], hsa_sb[:, sl],
                                 start=True, stop=True)
                nc.scalar.activation(wt_sb[:, sl], pw[:, :], AF.Tanh)
                pbc = pp_attn.tile([D, 512], F32, name=f"ps_bc{nt}",
                                   tag="attn")
                nc.tensor.matmul(pbc[:, :], ones10[:, :], wt_sb[0:1, sl],
                                 start=True, stop=True)
                nc.vector.tensor_mul(ys_scr[:, :], hsa_sb[:, sl], pbc[:, :])
                nc.vector.reduce_sum(ys_part[:, nt:nt + 1], ys_scr[:, :],
                                     axis=AX.X)
            nc.vector.reduce_sum(ppro[:, :], ys_part[:, :], axis=AX.X)
            ppro2 = cp.tile([D, 1], F32)
            nc.scalar.activation(ppro2[:, :], ppro[:, :], AF.Copy,
                                 scale=1.0 / L)
            nc.sync.dma_start(catv[D:2 * D, 0:1], ppro2[:, :])

            # ---------------- output MLP ------------------------------------
            for j in range(LO):
                pm = pp_attn.tile([2 * D, 1], F32, name=f"ps_m{j}", tag="attn")
                nc.tensor.matmul(pm[:, :], wo_sb[:, 2 * D * j:2 * D * (j + 1)],
                                 catv[:, j:j + 1], start=True, stop=True)
                nc.scalar.activation(catv[0:2 * D, j + 1:j + 2], pm[:, :],
                                     AF.Relu)
            pf = pp_attn.tile([2, 1], F32, name="ps_f", tag="attn")
            nc.tensor.matmul(pf[:, :], wi_sb[:, :], catv[:, LO:LO + 1],
                             start=True, stop=True)
            nc.scalar.activation(res_sb[:, :], pf[:, :], AF.Copy)
            nc.sync.dma_start(out_d[:, :], res_sb[:, :])

    nc.compile()
    return nc


def _prep_inputs(inputs):
    fp = np.asarray(inputs["fingerprints"]).astype(np.int64)
    A = np.ascontiguousarray(np.asarray(inputs["adjacency"], dtype=np.float32))
    words = np.asarray(inputs["words"]).astype(np.int64)
    emb_fp = np.asarray(inputs["emb_fp"], dtype=np.float32)
    emb_word = np.asarray(inputs["emb_word"], dtype=np.float32)
    Wg_w = np.asarray(inputs["Wg_w"], dtype=np.float32)
    Wg_b = np.asarray(inputs["Wg_b"], dtype=np.float32)
    conv_w = np.asarray(inputs["conv_w"], dtype=np.float32)
    conv_b = np.asarray(inputs["conv_b"], dtype=np.float32)
    Wa_w = np.asarray(inputs["Wa_w"], dtype=np.float32)
    Wa_b = np.asarray(inputs["Wa_b"], dtype=np.float32)
    Wo_w = np.asarray(inputs["Wo_w"], dtype=np.float32)
    Wo_b = np.asarray(inputs["Wo_b"], dtype=np.float32)
    Wi_w = np.asarray(inputs["Wi_w"], dtype=np.float32)
    Wi_b = np.asarray(inputs["Wi_b"], dtype=np.float32)

    xs0 = emb_fp[fp]                                     # [N, D]
    ws0 = emb_word[words]                                # [L, D]

    A8 = A.astype(FP8_NP)
    shared = {}
    shared["WS0"] = np.zeros((D, LW), np.float32)
    shared["WS0"][:, PAD:PAD + L] = ws0.T

    shared["WG"] = np.stack(
        [np.concatenate([Wg_w[i].T, Wg_b[i][None, :]], 0) for i in range(LG)])

    hst = np.zeros((LC, KW * D + 1, D), np.float32)
    c_idx = np.arange(D)[:, None]
    d_idx = np.arange(D)[None, :]
    j = c_idx - d_idx + PAD                              # [c, d]
    valid = (j >= 0) & (j < KW)
    jc = np.clip(j, 0, KW - 1)
    for i in range(LC):
        w = conv_w[i, 0, 0]                              # [KW, KW] (s, j)
        for s in range(KW):
            hst[i, D * s:D * (s + 1), :] = np.where(valid, w[s][jc], 0.0)
        hst[i, KW * D, :] = conv_b[i]
    shared["HST"] = hst

    shared["WA"] = np.concatenate([Wa_w.T, Wa_b[None, :]], 0)
    shared["WO"] = np.stack(
        [np.concatenate([Wo_w[i].T, Wo_b[i][None, :]], 0) for i in range(LO)])
    shared["WI"] = np.concatenate([Wi_w.T, Wi_b[None, :]], 0)
    shared["SUM8"] = np.tile(np.eye(D, dtype=np.float32), (NCORES, 1))
    shared["ONES"] = np.ones((1, LW), np.float32)

    in_maps = []
    for c in range(NCORES):
        blk = slice(NB * c, NB * (c + 1))
        m = dict(shared)
        m["AT"] = np.ascontiguousarray(A8[blk].T)        # [N, NB] fp8
        xs_c = np.ones((D + 1, NB), np.float32)
        xs_c[0:D] = xs0[blk].T
        m["XS0"] = xs_c
        in_maps.append(m)
    return in_maps


def run(inputs, trace=False, reps=1):
    from concourse.bass_utils import run_bass_kernel_spmd
    key = ("nc", reps)
    if key not in _CACHE:
        _CACHE[key] = _build_nc(reps)
    in_maps = _prep_inputs(inputs)
    res = run_bass_kernel_spmd(
        _CACHE[key], in_maps, core_ids=list(range(NCORES)), trace=trace)
    out = np.asarray(res.results[0]["OUT"], dtype=np.float32).reshape(1, 2)
    return out, res


def kernel(**inputs) -> np.ndarray:
    out, _ = run(inputs, trace=False)
    return out
